# revision 3
# baseline (speedup 1.0000x reference)
"""DiffusionBlock TRN2 kernel: spectral diffusion + sparse COO gradient op +
MLP + residual LayerNorm, sharded over 8 NeuronCores by node rows.

Self-contained: hardcodes all shapes; builds + compiles a Bass program at
call time (specialized to the edge distribution), runs SPMD on cores 0-7.
"""
import sys
sys.path.insert(0, '/opt/trn_rl_repo')
import numpy as np
import concourse.mybir as mybir
from concourse.bass import Bass
from concourse.tile import TileContext
from concourse import bass_utils, library_config

dt = mybir.dt

# problem dims (hardcoded per contract)
N, C, K, G, E = 65536, 256, 128, 32, 2097152
LN_EPS = 1e-5
NCORES = 8
RPC = N // NCORES          # rows per core = 8192
GPC = G // NCORES          # graphs per core = 4
NPG = N // G               # nodes per graph = 2048
NBLK = RPC // 128          # 128-row blocks per core = 64
BPG = NPG // 128           # blocks per graph = 16
CALL_IDX = 1024            # idxs per dma_gather call (hw limit)
HALF = 32768               # int16 gather index limit


# ---------------------------------------------------------------- BIR fixups
_wspill = [0]


def _legalize_waits(nc):
    """This walrus accepts at most 1 sync-wait per instruction (2 for
    EventSemaphore). Spill extras into EventSemaphore insts inserted just
    before, same engine. Also run codegen_inst_isa_subclasses (Bacc does it,
    raw Bass doesn't) so extended-ISA insts get their raw words."""
    mybir.codegen_inst_isa_subclasses(nc)
    f = nc.m.functions[0]
    for bb in f.blocks:
        out = []
        changed = False
        for ins in bb.instructions:
            si = ins.sync_info
            cap = 2 if ins.opcode == 'EventSemaphore' else 1
            if si is not None and si.on_wait is not None and len(si.on_wait) > cap:
                waits = list(si.on_wait)
                keep, spill = waits[:cap], waits[cap:]
                while spill:
                    batch, spill = spill[:2], spill[2:]
                    _wspill[0] += 1
                    es = mybir.InstEventSemaphore(
                        name=f"WSPILL-{_wspill[0]}", ins=[], outs=[])
                    es.engine = ins.engine
                    es.sync_info = mybir.SyncInfo(on_wait=batch, on_update=[])
                    out.append(es)
                si.on_wait = keep
                changed = True
            out.append(ins)
        if changed:
            bb.instructions = out
    return nc


# ---------------------------------------------------------------- host prep
def _prepare(inputs):
    x = np.asarray(inputs["x"], np.float32)
    evals = np.asarray(inputs["evals_batch"], np.float32)
    evecs = np.asarray(inputs["evecs"], np.float32)
    mass = np.asarray(inputs["mass"], np.float32)
    row = np.asarray(inputs["row"]).astype(np.int64)
    col = np.asarray(inputs["col"]).astype(np.int64)
    vals = np.asarray(inputs["vals"], np.float32)
    t_params = np.asarray(inputs["t_params"], np.float32)
    grad_W = np.asarray(inputs["grad_W"], np.float32)
    grad_b = np.asarray(inputs["grad_b"], np.float32)
    W1 = np.asarray(inputs["W1"], np.float32)
    b1 = np.asarray(inputs["b1"], np.float32)
    W2 = np.asarray(inputs["W2"], np.float32)
    b2 = np.asarray(inputs["b2"], np.float32)
    ln_g = np.asarray(inputs["ln_g"], np.float32)
    ln_b = np.asarray(inputs["ln_b"], np.float32)

    x16_full = x.astype(np.float16)

    # fold grad_W / grad_b into the second half of W1 (host, fp64 for accuracy)
    W1a = W1[:, :C]
    W1b = W1[:, C:]
    Wfold = (W1b.astype(np.float64) @ grad_W.astype(np.float64)).astype(np.float32)
    b1f_np = b1 + (W1b.astype(np.float64) @ grad_b.astype(np.float64)).astype(np.float32)

    # decay[g,k,c] = exp(-|t_c| * max(ev_gk, 0))
    t = np.abs(t_params)
    ev = np.maximum(evals.reshape(G, K), 0.0)
    decay = np.exp(-ev[:, :, None] * t[None, None, :]).astype(np.float32)  # [G,K,C]

    em_full = (evecs * mass[:, None]).astype(np.float16)   # [N,K]
    ev16_full = evecs.astype(np.float16)

    # ---- edge partitioning by destination row ----
    core_of = row >> 13               # row // 8192
    order = np.argsort(core_of, kind='stable')
    # per (core, block) bucketing
    CLo = CHi = 0
    per_core = []
    for i in range(NCORES):
        sel = np.where(core_of == i)[0]
        r = row[sel] - i * RPC
        c_ = col[sel]
        v = vals[sel]
        blk = r >> 7
        lo = c_ < HALF
        # counts per block/half
        lists = []
        for b in range(NBLK):
            m = blk == b
            mlo = m & lo
            mhi = m & ~lo
            lists.append((np.where(mlo)[0], np.where(mhi)[0]))
            CLo = max(CLo, (len(lists[-1][0]) + 127) // 128)
            CHi = max(CHi, (len(lists[-1][1]) + 127) // 128)
        per_core.append((r, c_, v, lists))

    ncall_lo = (CLo + 7) // 8
    ncall_hi = (CHi + 7) // 8
    NCALL = ncall_lo + ncall_hi
    CT = CLo + CHi

    in_maps = []
    for i in range(NCORES):
        r, c_, v, lists = per_core[i]
        gidx = np.zeros((NBLK, 16, NCALL, 64), np.int16)
        meta = np.zeros((NBLK, 128, 2, CT), np.float32)
        meta[:, :, 0, :] = 255.0  # localrow padding: never matches iota
        for b in range(NBLK):
            for half, (idxs_half, base_call, nch) in enumerate(
                    ((lists[b][0], 0, CLo), (lists[b][1], ncall_lo, CHi))):
                cc = c_[idxs_half] - half * HALF
                rr = r[idxs_half] & 127
                vv = v[idxs_half]
                ne = len(cc)
                npad = nch * 128
                ccp = np.zeros(npad, np.int64)
                ccp[:ne] = cc
                rrp = np.full(npad, 255.0, np.float32)
                rrp[:ne] = rr
                vvp = np.zeros(npad, np.float32)
                vvp[:ne] = vv
                for ch in range(nch):
                    ct_global = half * CLo + ch
                    sl = slice(ch * 128, (ch + 1) * 128)
                    meta[b, :, 0, ct_global] = rrp[sl]
                    meta[b, :, 1, ct_global] = vvp[sl]
                    # gather wrapped idx layout: within call, idx j at
                    # [j%16, j//16]; chunk ch occupies idx positions
                    # (ch%8)*128 .. +128 of call base_call + ch//8
                    call = base_call + ch // 8
                    j0 = (ch % 8) * 128
                    jj = np.arange(128) + j0
                    gidx[b, jj % 16, call, jj // 16] = ccp[sl].astype(np.int16)
        gidx = np.tile(gidx, (1, 8, 1, 1)).reshape(NBLK, 128, NCALL, 64)

        sl_rows = slice(i * RPC, (i + 1) * RPC)
        sl_g = slice(i * GPC, (i + 1) * GPC)
        evT16 = np.ascontiguousarray(
            ev16_full[sl_rows].reshape(GPC, NPG, K).transpose(0, 2, 1)
        ).reshape(GPC * K, NPG)

        c16 = np.zeros((128, 1792), np.float16)
        off = 0
        W1aT = W1a.T.astype(np.float16)
        WfT = Wfold.T.astype(np.float16)
        for Wt in (W1aT, WfT):
            for k in range(2):
                for m in range(2):
                    c16[:, off:off + 128] = Wt[k * 128:(k + 1) * 128,
                                               m * 128:(m + 1) * 128]
                    off += 128
        W2T = W2.T.astype(np.float16)
        for k in range(2):
            c16[:, off:off + 256] = W2T[k * 128:(k + 1) * 128, :]
            off += 256
        c16[:, off:off + 128] = np.eye(128, dtype=np.float16)
        off += 128
        c16[:, off:off + 128] = np.broadcast_to(
            np.arange(128, dtype=np.float16), (128, 128))
        off += 128
        assert off == 1792

        c32 = np.zeros((128, 516), np.float32)
        c32[:, 0] = b1f_np[:128]
        c32[:, 1] = b1f_np[128:]
        c32[:, 2:258] = np.broadcast_to(ln_g, (128, C))
        c32[:, 258:514] = np.broadcast_to(ln_b, (128, C))
        c32[:, 514] = LN_EPS

        in_maps.append({
            "xf": x16_full,
            "x16": np.ascontiguousarray(x16_full[sl_rows]),
            "xr": np.ascontiguousarray(x[sl_rows] + b2[None, :]),
            "em16": np.ascontiguousarray(em_full[sl_rows]),
            "evT16": evT16,
            "decay": np.ascontiguousarray(decay[sl_g]),
            "gidx": gidx,
            "meta": meta,
            "c16": c16,
            "c32": c32,
        })
    return in_maps, CLo, CHi, ncall_lo, ncall_hi


# ---------------------------------------------------------------- program
def _build(CLo, CHi, ncall_lo, ncall_hi):
    CT = CLo + CHi
    NCALL = ncall_lo + ncall_hi
    nc = Bass(num_swdge_queues=4)
    xf_h = nc.dram_tensor("xf", [N, C], dt.float16, kind="ExternalInput")
    x16_h = nc.dram_tensor("x16", [RPC, C], dt.float16, kind="ExternalInput")
    xr_h = nc.dram_tensor("xr", [RPC, C], dt.float32, kind="ExternalInput")
    em_h = nc.dram_tensor("em16", [RPC, K], dt.float16, kind="ExternalInput")
    evT_h = nc.dram_tensor("evT16", [GPC * K, NPG], dt.float16, kind="ExternalInput")
    dec_h = nc.dram_tensor("decay", [GPC, K, C], dt.float32, kind="ExternalInput")
    gidx_h = nc.dram_tensor("gidx", [NBLK, 128, NCALL, 64], dt.int16,
                            kind="ExternalInput")
    meta_h = nc.dram_tensor("meta", [NBLK, 128, 2, CT], dt.float32,
                            kind="ExternalInput")
    c16_h = nc.dram_tensor("c16", [128, 1792], dt.float16, kind="ExternalInput")
    c32_h = nc.dram_tensor("c32", [128, 516], dt.float32, kind="ExternalInput")
    out_h = nc.dram_tensor("out", [RPC, C], dt.float32, kind="ExternalOutput")

    TS = mybir.AluOpType
    AF = mybir.ActivationFunctionType

    with TileContext(nc) as tc:
        nc.gpsimd.load_library(library_config.mlp)
        nidx = nc.gpsimd.to_reg(CALL_IDX)
        with tc.tile_pool(name="consts", bufs=1) as cp, \
             tc.tile_pool(name="spec", bufs=2) as sp, \
             tc.tile_pool(name="gathp", bufs=12) as gp, \
             tc.tile_pool(name="segp", bufs=2) as sg, \
             tc.tile_pool(name="mlp", bufs=2) as mp, \
             tc.tile_pool(name="ln", bufs=3) as lp, \
             tc.tile_pool(name="ps", bufs=1, space="PSUM") as pp:
            c16 = cp.tile([128, 1792], dt.float16)
            c32 = cp.tile([128, 516], dt.float32)
            nc.sync.dma_start(c16[:], c16_h[:, :])
            nc.sync.dma_start(c32[:], c32_h[:, :])
            W1aT = [[c16[:, (k * 2 + m) * 128:(k * 2 + m + 1) * 128]
                     for m in range(2)] for k in range(2)]
            WfT = [[c16[:, 512 + (k * 2 + m) * 128:512 + (k * 2 + m + 1) * 128]
                    for m in range(2)] for k in range(2)]
            W2T = [c16[:, 1024 + k * 256:1024 + (k + 1) * 256] for k in range(2)]
            ident = c16[:, 1536:1664]
            iota = c16[:, 1664:1792]
            b1f = [c32[:, m:m + 1] for m in range(2)]
            grep = c32[:, 2:258]
            brep = c32[:, 258:514]
            eps_ap = c32[:, 514:515]
            zero_ap = c32[:, 515:516]

            qn = [0]

            def gather_block(B):
                gidx_t = gp.tile([128, NCALL, 64], dt.int16, tag="gidx")
                nc.sync.dma_start(gidx_t[:], gidx_h[B])
                meta_t = gp.tile([128, 2, CT], dt.float32, tag="meta")
                nc.sync.dma_start(meta_t[:], meta_h[B])
                tiles = []
                for call in range(NCALL):
                    gt = gp.tile([128, 8, C], dt.float16, tag="gath")
                    src = xf_h[0:HALF, :] if call < ncall_lo else xf_h[HALF:N, :]
                    nc.gpsimd.dma_gather(gt[:], src, gidx_t[:, call, :],
                                         CALL_IDX, nidx, C,
                                         queue_num=qn[0] % 4)
                    qn[0] += 1
                    tiles.append(gt)
                return meta_t, tiles

            def seg_block(B, meta_t, tiles, segT_t):
                psg = pp.tile([128, C], dt.float32, tag="grad", bufs=2)
                for ci in range(CT):
                    call, slot = (ci // 8, ci % 8) if ci < CLo else \
                        (ncall_lo + (ci - CLo) // 8, (ci - CLo) % 8)
                    ohv = gp.tile([128, 128], dt.float16, tag="ohv", bufs=4)
                    nc.vector.tensor_scalar(
                        ohv[:], iota, meta_t[:, 0, ci:ci + 1],
                        meta_t[:, 1, ci:ci + 1], TS.is_equal, TS.mult)
                    nc.tensor.matmul(psg[:], ohv[:], tiles[call][:, slot, :],
                                     start=(ci == 0), stop=(ci == CT - 1))
                segNM = sg.tile([128, C], dt.float16, tag="segNM")
                nc.scalar.copy(segNM[:], psg[:])
                for h2 in range(2):
                    tp = pp.tile([128, 128], dt.float16, tag="tp", bufs=1)
                    nc.tensor.transpose(tp[:], segNM[:, h2 * 128:(h2 + 1) * 128],
                                        ident)
                    nc.vector.tensor_copy(
                        segT_t[:, h2, (B % 4) * 128:(B % 4 + 1) * 128], tp[:])

            def mlp_group(B, segT_t, diffT):
                # nodes n0..n0+512 within this core
                n0 = (B - 3) * 128
                gslice = slice(n0 - (n0 // NPG) * NPG, n0 - (n0 // NPG) * NPG + 512)
                hT = mp.tile([128, 2, 512], dt.float16, tag="hT")
                for m in range(2):
                    ph = pp.tile([128, 512], dt.float32, tag="h", bufs=2)
                    first = True
                    for k in range(2):
                        nc.tensor.matmul(ph[:], W1aT[k][m], diffT[k][:, gslice],
                                         start=first, stop=False)
                        first = False
                    for k in range(2):
                        nc.tensor.matmul(ph[:], WfT[k][m], segT_t[:, k, :],
                                         start=False, stop=(k == 1))
                    nc.scalar.activation(hT[:, m, :], ph[:], AF.Relu,
                                         bias=b1f[m], scale=1.0)
                for tt in range(4):
                    nt = B - 3 + tt   # node tile index (128 rows)
                    py = pp.tile([128, C], dt.float32, tag="y", bufs=1)
                    for k in range(2):
                        nc.tensor.matmul(py[:], hT[:, k, tt * 128:(tt + 1) * 128],
                                         W2T[k], start=(k == 0), stop=(k == 1))
                    xrt = lp.tile([128, C], dt.float32, tag="xr")
                    nc.sync.dma_start(xrt[:], xr_h[nt * 128:(nt + 1) * 128, :])
                    y = lp.tile([128, C], dt.float32, tag="y")
                    nc.vector.tensor_add(y[:], py[:], xrt[:])
                    nsum = lp.tile([128, 1], dt.float32, tag="s0")
                    nc.vector.tensor_reduce(nsum[:], y[:], mybir.AxisListType.X,
                                            TS.add, negate=True)
                    nmu = lp.tile([128, 1], dt.float32, tag="s1")
                    nc.vector.tensor_scalar_mul(nmu[:], nsum[:], 1.0 / C)
                    sq = lp.tile([128, C], dt.float32, tag="sq")
                    sqs = lp.tile([128, 1], dt.float32, tag="s2")
                    nc.scalar.activation(sq[:], y[:], AF.Square,
                                         bias=zero_ap, accum_out=sqs[:])
                    ex2 = lp.tile([128, 1], dt.float32, tag="s3")
                    nc.vector.tensor_scalar_mul(ex2[:], sqs[:], 1.0 / C)
                    mu2 = lp.tile([128, 1], dt.float32, tag="s4")
                    nc.vector.tensor_mul(mu2[:], nmu[:], nmu[:])
                    var = lp.tile([128, 1], dt.float32, tag="s5")
                    nc.vector.tensor_sub(var[:], ex2[:], mu2[:])
                    sd = lp.tile([128, 1], dt.float32, tag="s6")
                    nc.scalar.activation(sd[:], var[:], AF.Sqrt, bias=eps_ap)
                    rstd = lp.tile([128, 1], dt.float32, tag="s7")
                    nc.vector.reciprocal(rstd[:], sd[:])
                    yn = lp.tile([128, C], dt.float32, tag="yn")
                    nc.vector.tensor_scalar(yn[:], y[:], nmu[:], rstd[:],
                                            TS.add, TS.mult)
                    yg = lp.tile([128, C], dt.float32, tag="yg")
                    nc.vector.tensor_mul(yg[:], yn[:], grep)
                    ot = lp.tile([128, C], dt.float32, tag="ot")
                    nc.vector.tensor_add(ot[:], yg[:], brep)
                    nc.sync.dma_start(out_h[nt * 128:(nt + 1) * 128, :], ot[:])

            for g in range(GPC):
                em_t = sp.tile([128, 16, K], dt.float16, tag="em")
                nc.sync.dma_start(
                    em_t[:], em_h[g * NPG:(g + 1) * NPG, :].rearrange(
                        "(j p) k -> p j k", p=128))
                xg_t = sp.tile([128, 16, C], dt.float16, tag="xg")
                nc.sync.dma_start(
                    xg_t[:], x16_h[g * NPG:(g + 1) * NPG, :].rearrange(
                        "(j p) k -> p j k", p=128))
                evT_t = sp.tile([128, NPG], dt.float16, tag="evT")
                nc.sync.dma_start(evT_t[:], evT_h[g * K:(g + 1) * K, :])
                dec_t = sp.tile([128, C], dt.float32, tag="dec")
                nc.sync.dma_start(dec_t[:], dec_h[g])

                pxs = pp.tile([128, C], dt.float32, tag="xspec", bufs=1)
                for j in range(16):
                    nc.tensor.matmul(pxs[:], em_t[:, j, :], xg_t[:, j, :],
                                     start=(j == 0), stop=(j == 15))
                xsd = sp.tile([128, C], dt.float16, tag="xsd")
                nc.vector.tensor_mul(xsd[:], pxs[:], dec_t[:])

                diffT = [sp.tile([128, NPG], dt.float16, tag=f"diffT{h2}",
                                 name=f"diffT{h2}")
                         for h2 in range(2)]
                for h2 in range(2):
                    for j in range(4):
                        pd = pp.tile([128, 512], dt.float32, tag="diff", bufs=1)
                        nc.tensor.matmul(pd[:],
                                         xsd[:, h2 * 128:(h2 + 1) * 128],
                                         evT_t[:, j * 512:(j + 1) * 512],
                                         start=True, stop=True)
                        nc.scalar.copy(diffT[h2][:, j * 512:(j + 1) * 512], pd[:])

                for b in range(BPG):
                    B = g * BPG + b
                    if b % 4 == 0:
                        segT_t = sg.tile([128, 2, 512], dt.float16, tag="segT")
                    meta_t, tiles = gather_block(B)
                    seg_block(B, meta_t, tiles, segT_t)
                    if b % 4 == 3:
                        mlp_group(B, segT_t, diffT)
    _legalize_waits(nc)
    return nc


# ---------------------------------------------------------------- numpy emu
def emulate_numpy(inputs):
    """Numpy emulation of the device dataflow (fp16 where the device uses
    fp16) — validates host prep + layout logic without hardware."""
    in_maps, CLo, CHi, ncall_lo, ncall_hi = _prepare(inputs)
    CT = CLo + CHi
    outs = []
    for i in range(NCORES):
        m = in_maps[i]
        xf = m["xf"].astype(np.float32)
        out = np.zeros((RPC, C), np.float32)
        # spectral
        em = m["em16"].astype(np.float32).reshape(GPC, NPG, K)
        xg = m["x16"].astype(np.float32).reshape(GPC, NPG, C)
        evT = m["evT16"].astype(np.float32).reshape(GPC, K, NPG)
        xspec = np.einsum('gnk,gnc->gkc', em, xg)
        xsd = (xspec * m["decay"]).astype(np.float16).astype(np.float32)
        diff = np.einsum('gkn,gkc->gnc', evT, xsd).reshape(RPC, C)
        # segment sum
        seg = np.zeros((RPC, C), np.float32)
        meta = m["meta"]
        gidx = m["gidx"]
        for b in range(NBLK):
            for ci in range(CT):
                call, slot = (ci // 8, ci % 8) if ci < CLo else \
                    (ncall_lo + (ci - CLo) // 8, (ci - CLo) % 8)
                jj = np.arange(128) + slot * 128
                idxs = gidx[b, jj % 16, call, jj // 16].astype(np.int64)
                base = 0 if call < ncall_lo else HALF
                g_rows = xf[idxs + base]                       # [128, C] fp16 vals
                lrow = meta[b, :, 0, ci]
                val = meta[b, :, 1, ci]
                ohv = (lrow[:, None] ==
                       np.arange(128)[None, :]).astype(np.float32) \
                    * val.astype(np.float16).astype(np.float32)[:, None]
                seg[b * 128:(b + 1) * 128] += ohv.T @ (
                    g_rows.astype(np.float16).astype(np.float32))
        segT = seg.astype(np.float16).astype(np.float32)
        diffT = diff.astype(np.float16).astype(np.float32)
        # MLP via folded weights
        W1aT = np.zeros((C, C), np.float32)
        WfT = np.zeros((C, C), np.float32)
        c16 = m["c16"].astype(np.float32)
        for k in range(2):
            for mm_ in range(2):
                W1aT[k * 128:(k + 1) * 128, mm_ * 128:(mm_ + 1) * 128] = \
                    c16[:, (k * 2 + mm_) * 128:(k * 2 + mm_ + 1) * 128]
                WfT[k * 128:(k + 1) * 128, mm_ * 128:(mm_ + 1) * 128] = \
                    c16[:, 512 + (k * 2 + mm_) * 128:512 + (k * 2 + mm_ + 1) * 128]
        W2T = np.concatenate([c16[:, 1024:1280], c16[:, 1280:1536]], 0)
        b1f = np.concatenate([m["c32"][:, 0], m["c32"][:, 1]])
        h = np.maximum(diffT @ W1aT + segT @ WfT + b1f, 0.0)
        h = h.astype(np.float16).astype(np.float32)
        y = m["xr"] + h @ W2T
        mu = y.mean(-1, keepdims=True)
        var = (y * y).mean(-1, keepdims=True) - mu * mu
        g_ = m["c32"][0, 2:258]
        b_ = m["c32"][0, 258:514]
        out = (y - mu) / np.sqrt(var + LN_EPS) * g_ + b_
        outs.append(out)
    return np.concatenate(outs, 0)


# ---------------------------------------------------------------- entry
def kernel(**inputs):
    in_maps, CLo, CHi, ncall_lo, ncall_hi = _prepare(inputs)
    nc = _build(CLo, CHi, ncall_lo, ncall_hi)
    res = bass_utils.run_bass_kernel_spmd(nc, in_maps,
                                          core_ids=list(range(NCORES)))
    return np.concatenate([res.results[i]["out"] for i in range(NCORES)], 0)
